# revision 5
# baseline (speedup 1.0000x reference)
"""Trainium2 Bass kernel for nn_BatchProgramCC (siamese program classifier).

Network (per side): embed tokens -> per-statement conv (Wc) + tanh + masked
max over tokens -> bidirectional GRU over statements -> residual -> max over
time. Head: softmax(h2l @ |lvec - rvec|).

Distribution: pure data-parallel over the batch (B=32) across 8 NeuronCores
(4 program-pairs per core); weights/embedding replicated. Each core runs an
identical NEFF on its own batch shard; the host concatenates the 8 output
shards. No collectives.

Device-side dataflow per core:
  * embedding gather via the DMA-gather-transpose path (two passes over a
    zero-row-padded bf16 table to cover V=50000 with int16 indices); invalid
    tokens re-gather the statement's first token so the per-statement max is
    unchanged (no -inf masking needed anywhere).
  * Wc projection as bf16 matmuls (E on partitions), per-statement token max
    via grouped reduce_max straight out of PSUM, tanh(+bias) on ACT,
    statement-validity mask multiply.
  * xw = Wih @ enc precomputed for all steps/gates; the GRU runs 128
    sequential steps with fwd+bwd and both sides merged into one 16-lane
    chain ([H on partitions, lanes on free]); per step an identity-matmul
    injects xw into PSUM, 6 small matmuls accumulate Whh h, sigmoid/tanh on
    ACT, gate algebra on DVE.
  * residual + max-pool over time, |l - r|, 2-class softmax via sigmoid.
"""

import os
import numpy as np
import ml_dtypes

# ---------------------------------------------------------------- sizes ----
V, E, D, H, L = 50000, 128, 256, 128, 2
B, S, T = 32, 128, 32
NCORES = 8
PB = B // NCORES            # programs per core = 4
NLANE = 2 * PB              # sequences per direction per core = 8 (side-major)
NSTMT = NLANE * S           # statements per core = 1024
NTOK = NSTMT * T            # tokens per core = 32768
W2 = 2 * NLANE              # lanes across both directions = 16
SPLIT = 32766               # ids < SPLIT go to gather pass A
PTAB_ROWS = 2 + V           # 50002 (two zero rows)
NCHUNK = 8                  # gather chunks
CTOK = NTOK // NCHUNK       # tokens per chunk = 4096

BF16 = ml_dtypes.bfloat16

_cache = {}


# ------------------------------------------------------------ device IR ----
def _build_program(nvc=NSTMT, gq=None):
    from contextlib import ExitStack
    import concourse.mybir as mybir
    import concourse.tile as tile
    from concourse import bacc
    from concourse.masks import make_identity

    dt = mybir.dt
    Alu = mybir.AluOpType
    Act = mybir.ActivationFunctionType

    nc = bacc.Bacc("TRN2", target_bir_lowering=False, debug=False,
                   num_devices=NCORES)

    ntokc = nvc * T          # compacted token count (statement-compacted)
    nchunk = ntokc // CTOK   # gather chunks of 4096 tokens / 128 statements

    # ---- DRAM tensors (per-core views; same names on every core) ----
    ptab = nc.dram_tensor("ptab", [PTAB_ROWS, E], dt.bfloat16, kind="ExternalInput")
    idxag = nc.dram_tensor("idxag", [128, ntokc // 16], dt.int16,
                           kind="ExternalInput")
    idxbg = nc.dram_tensor("idxbg", [128, ntokc // 16], dt.int16,
                           kind="ExternalInput")
    dcidx = nc.dram_tensor("dcidx", [128, NSTMT // 16], dt.int16,
                           kind="ExternalInput")
    plens = nc.dram_tensor("plens", [NLANE], dt.int32, kind="ExternalInput")
    wcT = nc.dram_tensor("wcT", [E, 2, 128], dt.bfloat16, kind="ExternalInput")
    wcb = nc.dram_tensor("wcb", [128, 2], dt.float32, kind="ExternalInput")
    wihT = nc.dram_tensor("wihT", [2, 2, 128, 3 * H], dt.bfloat16, kind="ExternalInput")
    whhT = nc.dram_tensor("whhT", [2, H, 3 * H], dt.bfloat16, kind="ExternalInput")
    bih3 = nc.dram_tensor("bih3", [2, 128, 3], dt.float32, kind="ExternalInput")
    bhh3 = nc.dram_tensor("bhh3", [2, 128, 3], dt.float32, kind="ExternalInput")
    h2lT = nc.dram_tensor("h2lT", [2, 128, 1], dt.float32, kind="ExternalInput")
    h2lb = nc.dram_tensor("h2lb", [1, L], dt.float32, kind="ExternalInput")
    out_d = nc.dram_tensor("probs", [PB, L], dt.float32, kind="ExternalOutput")

    cut = os.environ.get("BPCC_CUT", "")
    with tile.TileContext(nc) as tc, ExitStack() as ctx:
        persist = ctx.enter_context(tc.tile_pool(name="persist", bufs=1))
        dram = ctx.enter_context(tc.tile_pool(name="dram", bufs=1, space="DRAM"))

        # ---------------- persistent SBUF buffers ----------------
        def ptile(shape, dtype, name):
            return persist.tile(shape, dtype, tag=name, name=name)

        w_wcT = ptile([E, 2, 128], dt.bfloat16, "w_wcT")
        w_wcb = ptile([128, 2], dt.float32, "w_wcb")
        w_wihT = ptile([128, 2, 2, 3 * H], dt.bfloat16, "w_wihT")
        w_whhT = ptile([128, 2, 3 * H], dt.bfloat16, "w_whhT")
        b_ih = ptile([128, 2, 3], dt.float32, "b_ih")
        b_hh = ptile([128, 2, 3], dt.float32, "b_hh")
        b_xw = ptile([128, 2, 3], dt.float32, "b_xw")
        w_h2lT = ptile([128, 2, 1], dt.float32, "w_h2lT")
        w_h2lb = ptile([1, L], dt.float32, "w_h2lb")
        pl_sb = ptile([NLANE, 1], dt.int32, "pl_sb")

        idxAr = ptile([128, ntokc // 16], dt.int16, "idxAr")
        idxBr = ptile([128, ntokc // 16], dt.int16, "idxBr")
        dc_sb = ptile([128, NSTMT // 16], dt.int16, "dc_sb")

        ident = ptile([128, 128], dt.bfloat16, "ident")
        smask = ptile([128, NSTMT], dt.bfloat16, "smask")
        stmt_pre = ptile([128, 2, nvc], dt.float32, "stmt_pre")
        encC = ptile([128, 2, nvc], dt.float32, "encC")
        encT = ptile([128, 2, NSTMT], dt.bfloat16, "encT")
        # xw per step: 64 cols = [r_f8 r_b8 | z_f8 z_b8 | (bhh_n,xw_n)x16]
        # (backward direction is stored step-reversed; the n-gate block is
        # interleaved (bhh_n, xw_n) pairs per lane for the psum inject; the
        # z block is NEGATED -- z-gate weights/bias are negated host-side so
        # one sigmoid yields [r | 1-z] at once)
        xw48 = ptile([128, S, 8 * NLANE], dt.bfloat16, "xw48")
        # h stored interleaved: [., t, lane, 1] = h; [., t, lane, 0] = n junk
        outbuf = ptile([128, S + 1, 2 * NLANE, 2], dt.bfloat16, "outbuf")
        # [0, r]x16 pairs then [0, zc]x16 pairs (evens stay zero)
        rzpat0 = ptile([128, 2 * W2, 2], dt.float32, "rzpat0")
        rzpat1 = ptile([128, 2 * W2, 2], dt.float32, "rzpat1")
        bn_z = ptile([128, 2], dt.float32, "bn_z")
        mxall = ptile([128, W2], dt.float32, "mxall")
        ad = ptile([128, 2, PB], dt.float32, "ad")
        probs_sb = ptile([1, 2 * PB], dt.float32, "probs_sb")

        # ---------------- weight / input loads ----------------
        nc.sync.dma_start(w_wcT[:], wcT[:])
        nc.sync.dma_start(w_wcb[:], wcb[:])
        nc.sync.dma_start(w_wihT[:], wihT[:].rearrange("d k p g -> p d k g"))
        nc.sync.dma_start(w_whhT[:], whhT[:].rearrange("d p g -> p d g"))
        nc.sync.dma_start(b_ih[:], bih3[:].rearrange("d p g -> p d g"))
        nc.sync.dma_start(b_hh[:], bhh3[:].rearrange("d p g -> p d g"))
        nc.sync.dma_start(w_h2lT[:], h2lT[:].rearrange("k p l -> p k l"))
        nc.sync.dma_start(w_h2lb[:], h2lb[:])
        nc.sync.dma_start(pl_sb[:], plens[:].rearrange("(p o) -> p o", o=1))
        nc.sync.dma_start(idxAr[:], idxag[:])
        nc.sync.dma_start(idxBr[:], idxbg[:])
        nc.sync.dma_start(dc_sb[:], dcidx[:])

        make_identity(nc, ident[:])

        # combined bias for the xw fold: r/z get bih+bhh, n gets bih only
        nc.vector.tensor_tensor(b_xw[:], b_ih[:], b_hh[:], Alu.add)
        nc.vector.tensor_copy(b_xw[:, :, 2], b_ih[:, :, 2])
        # negated z bias (the z preact is stored negated)
        nc.vector.tensor_scalar_mul(bn_z[:], b_xw[:, :, 1], -1.0)

        # ---------------- statement-validity mask ----------------
        scratch = ctx.enter_context(tc.tile_pool(name="scratch", bufs=1))
        # msp[lane, s] = (plen[lane] >= S - s); bounce through DRAM to
        # replicate across the 128 D-partitions.
        thr = scratch.tile([NLANE, S], dt.int32, tag="thr")
        nc.gpsimd.iota(thr[:], pattern=[[-1, S]], base=S, channel_multiplier=0)
        thr_f = scratch.tile([NLANE, S], dt.float32, tag="thr_f")
        nc.vector.tensor_copy(thr_f[:], thr[:])
        pl_f = scratch.tile([NLANE, 1], dt.float32, tag="pl_f")
        nc.vector.tensor_copy(pl_f[:], pl_sb[:])
        msp_bf = scratch.tile([NLANE, S], dt.bfloat16, tag="msp_bf")
        nc.vector.tensor_single_scalar(out=msp_bf[:], in_=thr_f[:],
                                       scalar=pl_f[:], op=Alu.is_le)
        smask_d = dram.tile([NLANE, S], dt.bfloat16, tag="smask_d")
        nc.sync.dma_start(smask_d[:], msp_bf[:])
        nc.sync.dma_start(
            smask[:],
            smask_d[:].rearrange("l s -> (l s)").unsqueeze(0)
            .broadcast_to([128, NSTMT]))

        # ---------------- embed: gather + Wc + token-max (compacted) ----
        with tc.tile_pool(name="gx", bufs=3) as gx, \
             tc.tile_pool(name="pemb", bufs=6, space="PSUM") as pemb:
            for j in range(nchunk):
                xa = gx.tile([128, 1, CTOK], dt.bfloat16, tag="xa")
                xb = gx.tile([128, 1, CTOK], dt.bfloat16, tag="xb")
                gsl = slice((CTOK // 16) * j, (CTOK // 16) * (j + 1))
                nc.gpsimd.dma_gather(
                    out_ap=xa[:], in_ap=ptab[0:SPLIT + 1, :],
                    idxs_ap=idxAr[:, gsl],
                    num_idxs=CTOK, num_idxs_reg=CTOK, elem_size=E,
                    transpose=True, single_packet=False)
                nc.gpsimd.dma_gather(
                    out_ap=xb[:], in_ap=ptab[SPLIT + 1:PTAB_ROWS, :],
                    idxs_ap=idxBr[:, gsl],
                    num_idxs=CTOK, num_idxs_reg=CTOK, elem_size=E,
                    transpose=True, single_packet=False)
                if cut == "gather":
                    continue
                for dh in range(2):
                    for mt in range(CTOK // 512):
                        ps = pemb.tile([128, 512], dt.float32, tag="pe")
                        sl = slice(512 * mt, 512 * (mt + 1))
                        nc.tensor.matmul(ps[:], w_wcT[:, dh, :],
                                         xa[:, 0, sl], start=True, stop=False)
                        nc.tensor.matmul(ps[:], w_wcT[:, dh, :],
                                         xb[:, 0, sl], start=False, stop=True)
                        c0 = j * (CTOK // T) + mt * 16
                        nc.vector.tensor_reduce(
                            out=stmt_pre[:, dh, c0:c0 + 16],
                            in_=ps[:].rearrange("p (g t) -> p g t", t=T),
                            axis=mybir.AxisListType.X, op=Alu.max)

        # ------- enc = smask * decompact(tanh(pre + bias)) -------
        if cut != "gather":
            for dh in range(2):
                nc.scalar.activation(encC[:, dh, :], stmt_pre[:, dh, :],
                                     Act.Tanh, bias=w_wcb[:, dh:dh + 1],
                                     scale=1.0)
                encF = scratch.tile([128, NSTMT], dt.float32, tag=f"encF{dh}")
                nc.gpsimd.ap_gather(
                    out_ap=encF[:].rearrange("p (v o) -> p v o", o=1),
                    in_ap=encC[:, dh, :].rearrange("p (v o) -> p v o", o=1),
                    idxs_ap=dc_sb[:], channels=128, num_elems=nvc, d=1,
                    num_idxs=NSTMT)
                nc.vector.tensor_tensor(encT[:, dh, :], encF[:], smask[:],
                                        Alu.mult)

        # ---------------- xw precompute ----------------
        # encT cols are lane-major: col = lane*S + s
        with tc.tile_pool(name="pxw", bufs=4, space="PSUM") as pxw:
          if cut not in ("embed", "gather"):
            for d in range(2):
                for g in range(3):
                    for n2 in range(NSTMT // 512):
                        ps = pxw.tile([128, 512], dt.float32, tag="pxw")
                        for kb in range(2):
                            nc.tensor.matmul(
                                ps[:],
                                w_wihT[:, d, kb, g * H:(g + 1) * H],
                                encT[:, kb, 512 * n2:512 * (n2 + 1)],
                                start=(kb == 0), stop=(kb == 1))
                        # psum cols = (lane, s) lane-major; lanes n2*4..n2*4+4
                        l0 = d * NLANE + 4 * n2
                        if g < 2:
                            dst = xw48[:, :, g * W2 + l0:g * W2 + l0 + 4]
                        else:
                            # n gate: odd cols of the interleaved pair block
                            dst = xw48[:, :, 2 * W2:4 * W2].rearrange(
                                "p s (l two) -> p s l two", two=2)[
                                :, :, l0:l0 + 4, 1]
                        if d == 1:   # backward direction: store s-reversed
                            dst = dst[:, ::-1, :]
                        dst = dst.transpose([0, 2, 1])  # iterate (lane, s)
                        src = ps[:].rearrange("p (l s) -> p l s", s=S)
                        if g in (0, 2):
                            nc.scalar.activation(dst, src, Act.Identity,
                                                 bias=b_xw[:, d, g:g + 1],
                                                 scale=1.0)
                        else:
                            # z gate stored negated: -(xw_z + b_z), on DVE
                            # to unload ACT during the xw phase
                            nc.vector.tensor_scalar(
                                out=dst, in0=src,
                                scalar1=b_xw[:, d, 1:2], scalar2=-1.0,
                                op0=Alu.add, op1=Alu.mult)
            # bhh_n broadcast into the even cols of the n-gate block
            for d in range(2):
                pv = xw48[:, :, 2 * W2:4 * W2].rearrange(
                    "p s (l two) -> p s l two", two=2)
                nc.scalar.activation(pv[:, :, d * NLANE:(d + 1) * NLANE, 0],
                                     pv[:, :, d * NLANE:(d + 1) * NLANE, 1],
                                     Act.Identity, bias=b_hh[:, d, 2:3],
                                     scale=0.0)

        # ---------------- GRU: 128 sequential steps ----------------
        # Both directions run in one instruction stream: 16 lanes
        # (fwd 0:8, bwd 8:16).  Per step the psum holds
        #   [r-preact 16 | z-preact 16 | (pn', xw_n) interleaved 32]
        # with pn' = Whh_n h + bhh_n.  One DVE scan computes
        # y = r*pn' + xw_n (pairwise reset via op0=mult with 0), tanh gives
        # n written into the step's junk column, an in-place subtract turns
        # it into n-h, and a second scan computes h' = h + zc*(n-h)
        # (zc = 1-z straight from a scale=-1 sigmoid).  The running
        # residual+time-max is folded in every BLK steps so no tail
        # reduction over the full history is needed.
        nc.vector.memset(outbuf[:, 0, :, :], 0.0)
        nc.vector.memset(rzpat0[:], 0.0)
        nc.vector.memset(rzpat1[:], 0.0)
        nc.vector.memset(mxall[:], -3e38)
        BLK = 8
        with tc.tile_pool(name="pgru", bufs=3, space="PSUM") as pgru, \
             tc.tile_pool(name="gsb", bufs=3) as gsb:
          if cut not in ("embed", "xw", "gather"):
            NL = NLANE
            nsteps = int(os.environ.get("BPCC_GRUS", S))
            for t in range(nsteps):
                rzpat = rzpat0 if t % 2 == 0 else rzpat1
                hf = outbuf[:, t, 0:NL, 1]
                hb = outbuf[:, t, NL:W2, 1]
                pp = pgru.tile([128, 4 * W2], dt.float32, tag="pp")
                pnv = pp[:, 2 * W2:4 * W2].rearrange(
                    "p (l two) -> p l two", two=2)
                # one inject opens all three regions (r | -z | (bhh,xw_n));
                # each region is closed by its own Whh matmuls
                nc.tensor.matmul(pp[:], ident[:], xw48[:, t, :],
                                 start=True, stop=False)
                nc.tensor.matmul(pp[:, 0:NL], w_whhT[:, 0, 0:H], hf,
                                 start=False, stop=False)
                nc.tensor.matmul(pp[:, NL:W2], w_whhT[:, 1, 0:H], hb,
                                 start=False, stop=False)
                nc.tensor.matmul(pp[:, W2:W2 + NL], w_whhT[:, 0, H:2 * H],
                                 hf, start=False, stop=False)
                nc.tensor.matmul(pp[:, W2 + NL:2 * W2], w_whhT[:, 1, H:2 * H],
                                 hb, start=False, stop=False)
                nc.tensor.matmul(pnv[:, 0:NL, 0], w_whhT[:, 0, 2 * H:3 * H],
                                 hf, start=False, stop=False)
                nc.tensor.matmul(pnv[:, NL:W2, 0], w_whhT[:, 1, 2 * H:3 * H],
                                 hb, start=False, stop=True)
                # one sigmoid: [r | 1-z] -> rzpat odds (z block is negated)
                nc.scalar.activation(rzpat[:, :, 1], pp[:, 0:2 * W2],
                                     Act.Sigmoid)
                # y scan: odd cols get r*pn' + xw_n
                yb = pgru.tile([128, 2 * W2], dt.float32, tag="yb")
                nc.vector.tensor_tensor_scan(
                    yb[:], rzpat[:, 0:W2, :].rearrange("p l two -> p (l two)"),
                    pp[:, 2 * W2:4 * W2], 0.0, op0=Alu.mult, op1=Alu.add)
                ybv = yb[:].rearrange("p (l two) -> p l two", two=2)
                # n into the junk column of step t, then in-place n-h
                nc.scalar.activation(outbuf[:, t, :, 0], ybv[:, :, 1],
                                     Act.Tanh)
                nc.vector.tensor_tensor(outbuf[:, t, :, 0],
                                        outbuf[:, t, :, 0],
                                        outbuf[:, t, :, 1], Alu.subtract)
                # blend scan: odd cols get h + zc*(n-h) = h'
                nc.vector.tensor_tensor_scan(
                    outbuf[:, t + 1, :, :].rearrange("p l two -> p (l two)"),
                    rzpat[:, W2:2 * W2, :].rearrange("p l two -> p (l two)"),
                    outbuf[:, t, :, :].rearrange("p l two -> p (l two)"),
                    0.0, op0=Alu.mult, op1=Alu.add)
                # in-loop residual + time-max every BLK steps (consumers
                # only -- fills DVE gaps, no serial-chain impact)
                if t % BLK == BLK - 1 and cut == "":
                    t0 = t + 1 - BLK
                    g8 = gsb.tile([128, BLK, W2], dt.float32, tag="g8")
                    for dh in range(2):
                        ebase = encT[:, dh, :].rearrange(
                            "p (l s) -> p s l", s=S)
                        if dh == 1:
                            ebase = ebase[:, ::-1, :]
                        nc.vector.tensor_tensor(
                            g8[:, :, dh * NL:(dh + 1) * NL],
                            outbuf[:, t0 + 1:t0 + 1 + BLK,
                                   dh * NL:(dh + 1) * NL, 1],
                            ebase[:, t0:t0 + BLK, :], Alu.add)
                    bm = gsb.tile([128, W2], dt.float32, tag="bm")
                    nc.vector.tensor_reduce(
                        out=bm[:], in_=g8[:].transpose([0, 2, 1]),
                        axis=mybir.AxisListType.X, op=Alu.max)
                    nc.vector.tensor_tensor(mxall[:], mxall[:], bm[:],
                                            Alu.max)

        # ---------------- residual + time max-pool + head ----------------
        with tc.tile_pool(name="tail", bufs=1) as tail, \
             tc.tile_pool(name="phead", bufs=1, space="PSUM") as phead:
          if cut in ("embed", "xw", "gru"):
            nc.sync.dma_start(out_d[:], stmt_pre[:PB, 0, 0:L])
          if cut == "":
            for dh in range(2):
                # |lvec - rvec|  (lanes 0..3 = side1, 4..7 = side2);
                # the time-max already lives in mxall from the in-loop fold
                nc.vector.tensor_tensor(
                    ad[:, dh, :], mxall[:, dh * NLANE:dh * NLANE + PB],
                    mxall[:, dh * NLANE + PB:(dh + 1) * NLANE], Alu.subtract)
                nc.scalar.activation(ad[:, dh, :], ad[:, dh, :], Act.Abs)

            # head: only the logit DIFFERENCE matters for the 2-class
            # softmax; the host pre-subtracts the two head rows so this is a
            # single 1-col matmul (result on one partition, no bounce).
            pl = phead.tile([1, PB], dt.float32, tag="ph")
            for dh in range(2):
                nc.tensor.matmul(pl[:], w_h2lT[:, dh, :], ad[:, dh, :],
                                 start=(dh == 0), stop=(dh == 1))
            bd = tail.tile([1, 2], dt.float32, tag="bd")
            nc.vector.tensor_tensor(bd[:, 0:1], w_h2lb[:, 0:1],
                                    w_h2lb[:, 1:2], Alu.subtract)
            nc.vector.tensor_tensor(bd[:, 1:2], w_h2lb[:, 1:2],
                                    w_h2lb[:, 0:1], Alu.subtract)
            # softmax over 2 classes == sigmoid of the logit difference;
            # write interleaved (prog-major) so the output DMA is contiguous
            pview = probs_sb[:].rearrange("o (p l) -> o p l", l=L)
            nc.scalar.activation(pview[:, :, 0], pl[:], Act.Sigmoid,
                                 bias=bd[:, 0:1], scale=1.0)
            nc.scalar.activation(pview[:, :, 1], pl[:], Act.Sigmoid,
                                 bias=bd[:, 1:2], scale=-1.0)
            nc.sync.dma_start(out_d[:].rearrange("p l -> (p l)").unsqueeze(0),
                              probs_sb[:])

    nc.compile()
    return nc


def _get_program(nvc=NSTMT):
    key = ("nc", nvc)
    if key not in _cache:
        _cache[key] = _build_program(nvc)
    return _cache[key]


# ------------------------------------------------------------- host side ----
def _prep_shared(inputs):
    emb = np.asarray(inputs["emb"], np.float32)
    pt = np.zeros((PTAB_ROWS, E), dtype=BF16)
    pt[1:SPLIT + 1] = emb[:SPLIT].astype(BF16)
    pt[SPLIT + 2:] = emb[SPLIT:].astype(BF16)

    wcT = np.ascontiguousarray(
        np.asarray(inputs["Wc_w"], np.float32).T).astype(BF16)
    wcT = wcT.reshape(E, 2, 128)
    wcb = np.ascontiguousarray(
        np.asarray(inputs["Wc_b"], np.float32).reshape(2, 128).T)

    wihT = np.stack([np.asarray(inputs[k], np.float32).T
                     for k in ("wih_f", "wih_b")])
    wihT = np.ascontiguousarray(wihT.reshape(2, 2, 128, 3 * H)).astype(BF16)
    whhT = np.stack([np.asarray(inputs[k], np.float32).T
                     for k in ("whh_f", "whh_b")])
    whhT[:, :, 128:256] *= -1.0     # z-gate negated (see device comment)
    whhT = np.ascontiguousarray(whhT).astype(BF16)          # [2, 128, 384]
    bih3 = np.ascontiguousarray(np.stack(
        [np.asarray(inputs[k], np.float32).reshape(3, 128).T
         for k in ("bih_f", "bih_b")]))
    bhh3 = np.ascontiguousarray(np.stack(
        [np.asarray(inputs[k], np.float32).reshape(3, 128).T
         for k in ("bhh_f", "bhh_b")]))
    h2ld = np.asarray(inputs["h2l_w"], np.float32)
    h2lT = np.ascontiguousarray(
        (h2ld[0] - h2ld[1]).reshape(2, 128, 1))
    h2lb = np.asarray(inputs["h2l_b"], np.float32).reshape(1, L)
    return dict(ptab=pt, wcT=wcT, wcb=wcb, wihT=wihT, whhT=whhT,
                bih3=bih3, bhh3=bhh3, h2lT=h2lT, h2lb=h2lb)


def _wrap16(flat, width):
    """[n] int -> [128, n//16]: idx i of 4096-chunk j sits at
    [(i%16) + 16k, (CTOK//16)*j + i//16] for every partition group k."""
    n = flat.shape[0]
    out = np.zeros((128, n // 16), np.int16)
    for j in range(n // width):
        blk = flat[width * j:width * (j + 1)].reshape(width // 16, 16).T
        for k in range(8):
            out[16 * k:16 * (k + 1), (width // 16) * j:
                (width // 16) * (j + 1)] = blk
    return out


def _balance(inputs):
    """Assign program pairs to cores, balancing total valid statements."""
    w = (np.asarray(inputs["prog_lens1"]).astype(np.int64) +
         np.asarray(inputs["prog_lens2"]).astype(np.int64))
    order = np.argsort(-w)
    sums = [0] * NCORES
    buckets = [[] for _ in range(NCORES)]
    for i in order:
        open_cores = [j for j in range(NCORES) if len(buckets[j]) < PB]
        c = min(open_cores, key=lambda j: sums[j])
        buckets[c].append(int(i))
        sums[c] += int(w[i])
    perm = [i for b in buckets for i in b]
    nvc = -(-max(sums) // 128) * 128
    return perm, nvc


def _prep_core(c, inputs, perm, nvc):
    progs = perm[PB * c:PB * (c + 1)]
    tk = np.stack([np.asarray(inputs["tokens1"])[progs],
                   np.asarray(inputs["tokens2"])[progs]])
    tk = tk.astype(np.int64).reshape(NLANE, S, T)       # lane = side*PB+prog
    tl = np.stack([np.asarray(inputs["token_lens1"])[progs],
                   np.asarray(inputs["token_lens2"])[progs]])
    tl = tl.astype(np.int64).reshape(NLANE, S)
    pl = np.stack([np.asarray(inputs["prog_lens1"])[progs],
                   np.asarray(inputs["prog_lens2"])[progs]])
    pl = pl.astype(np.int32).reshape(NLANE)

    # compacted valid-statement list (lane-major, s ascending)
    lanes, ss = np.nonzero(np.arange(S)[None, :] >= (S - pl[:, None]))
    nv = lanes.shape[0]
    # effective tokens with invalid slots re-reading the first token
    tkv = tk[lanes, ss]                                 # [nv, T]
    tlv = tl[lanes, ss]                                 # [nv]
    valid = np.arange(T)[None, :] < tlv[:, None]
    eff = np.where(valid, tkv, tkv[:, 0:1])             # [nv, T]
    effp = np.zeros((nvc, T), np.int64)
    effp[:nv] = eff
    idxa = np.where((effp < SPLIT) & (np.arange(nvc)[:, None] < nv),
                    effp + 1, 0)
    idxb = np.where((effp >= SPLIT) & (np.arange(nvc)[:, None] < nv),
                    effp - (SPLIT - 1), 0)
    # decompaction: dense (lane, s) -> compact slot (invalid -> 0, masked)
    dc = np.zeros(NSTMT, np.int64)
    dc[lanes * S + ss] = np.arange(nv)

    return dict(idxag=_wrap16(idxa.reshape(nvc * T), CTOK),
                idxbg=_wrap16(idxb.reshape(nvc * T), CTOK),
                dcidx=_wrap16(dc, NSTMT),
                plens=np.ascontiguousarray(pl))


def _make_in_maps(inputs):
    perm, nvc = _balance(inputs)
    shared = _prep_shared(inputs)
    in_maps = []
    for c in range(NCORES):
        m = dict(shared)
        m.update(_prep_core(c, inputs, perm, nvc))
        in_maps.append(m)
    return in_maps, perm, nvc


def kernel(**inputs):
    from concourse import bass_utils

    in_maps, perm, nvc = _make_in_maps(inputs)
    nc = _get_program(nvc)
    res = bass_utils.run_bass_kernel_spmd(nc, in_maps,
                                          core_ids=list(range(NCORES)))
    kernel.last_results = res
    out = np.concatenate([res.results[c]["probs"] for c in range(NCORES)],
                         axis=0)                        # [B, L] permuted
    full = np.zeros((B, L), np.float32)
    full[np.asarray(perm)] = out.astype(np.float32)
    return np.ascontiguousarray(full.reshape(B, L, 1))


kernel.last_results = None



# revision 11
# speedup vs baseline: 1.2880x; 1.2880x over previous
"""Trainium2 Bass kernel for nn_BatchProgramCC (siamese program classifier).

Network (per side): embed tokens -> per-statement conv (Wc) + tanh + masked
max over tokens -> bidirectional GRU over statements -> residual -> max over
time. Head: softmax(h2l @ |lvec - rvec|).

Distribution: pure data-parallel over the batch (B=32) across 8 NeuronCores
(4 program-pairs per core); weights/embedding replicated. Each core runs an
identical NEFF on its own batch shard; the host concatenates the 8 output
shards. No collectives.

Device-side dataflow per core:
  * embedding gather via the DMA-gather-transpose path (two passes over a
    zero-row-padded bf16 table to cover V=50000 with int16 indices); invalid
    tokens re-gather the statement's first token so the per-statement max is
    unchanged (no -inf masking needed anywhere).
  * Wc projection as bf16 matmuls (E on partitions), per-statement token max
    via grouped reduce_max straight out of PSUM, tanh(+bias) on ACT,
    statement-validity mask multiply.
  * xw = Wih @ enc precomputed for all steps/gates; the GRU runs 128
    sequential steps with fwd+bwd and both sides merged into one 16-lane
    chain ([H on partitions, lanes on free]); per step an identity-matmul
    injects xw into PSUM, 6 small matmuls accumulate Whh h, sigmoid/tanh on
    ACT, gate algebra on DVE.
  * residual + max-pool over time, |l - r|, 2-class softmax via sigmoid.
"""

import os
import numpy as np
import ml_dtypes

# ---------------------------------------------------------------- sizes ----
V, E, D, H, L = 50000, 128, 256, 128, 2
B, S, T = 32, 128, 32
NCORES = 8
PB = B // NCORES            # programs per core = 4
NLANE = 2 * PB              # sequences per direction per core = 8 (side-major)
NSTMT = NLANE * S           # statements per core = 1024
NTOK = NSTMT * T            # tokens per core = 32768
W2 = 2 * NLANE              # lanes across both directions = 16
SPLIT = 32766               # ids < SPLIT go to gather pass A
PTAB_ROWS = 2 + V           # 50002 (two zero rows)
NCHUNK = 8                  # gather chunks
CTOK = NTOK // NCHUNK       # tokens per chunk = 4096

BF16 = ml_dtypes.bfloat16

_cache = {}


# ------------------------------------------------------------ device IR ----
def _build_program(nvc=NSTMT, gq=None):
    from contextlib import ExitStack
    import concourse.mybir as mybir
    import concourse.tile as tile
    from concourse import bacc
    from concourse.masks import make_identity

    dt = mybir.dt
    Alu = mybir.AluOpType
    Act = mybir.ActivationFunctionType

    nc = bacc.Bacc("TRN2", target_bir_lowering=False, debug=False,
                   num_devices=NCORES)

    ntokc = nvc * T          # compacted token count (statement-compacted)
    nchunk = ntokc // CTOK   # gather chunks of 4096 tokens / 128 statements

    # ---- DRAM tensors (per-core views; same names on every core) ----
    ptab = nc.dram_tensor("ptab", [PTAB_ROWS, E], dt.bfloat16, kind="ExternalInput")
    idxag = nc.dram_tensor("idxag", [128, ntokc // 16], dt.int16,
                           kind="ExternalInput")
    idxbg = nc.dram_tensor("idxbg", [128, ntokc // 16], dt.int16,
                           kind="ExternalInput")
    dcidx = nc.dram_tensor("dcidx", [128, NSTMT // 16], dt.int16,
                           kind="ExternalInput")
    plens = nc.dram_tensor("plens", [NLANE], dt.int32, kind="ExternalInput")
    wcT = nc.dram_tensor("wcT", [E, 2, 128], dt.bfloat16, kind="ExternalInput")
    wcb = nc.dram_tensor("wcb", [128, 2], dt.float32, kind="ExternalInput")
    wihT = nc.dram_tensor("wihT", [2, 2, 128, 3 * H], dt.bfloat16, kind="ExternalInput")
    whhT = nc.dram_tensor("whhT", [2, H, 3 * H], dt.bfloat16, kind="ExternalInput")
    bih3 = nc.dram_tensor("bih3", [2, 128, 3], dt.float32, kind="ExternalInput")
    bhh3 = nc.dram_tensor("bhh3", [2, 128, 3], dt.float32, kind="ExternalInput")
    h2lT = nc.dram_tensor("h2lT", [2, 128, 1], dt.float32, kind="ExternalInput")
    h2lb = nc.dram_tensor("h2lb", [1, L], dt.float32, kind="ExternalInput")
    out_d = nc.dram_tensor("probs", [PB, L], dt.float32, kind="ExternalOutput")

    cut = os.environ.get("BPCC_CUT", "")
    with tile.TileContext(nc) as tc, ExitStack() as ctx:
        persist = ctx.enter_context(tc.tile_pool(name="persist", bufs=1))
        dram = ctx.enter_context(tc.tile_pool(name="dram", bufs=1, space="DRAM"))

        # ---------------- persistent SBUF buffers ----------------
        def ptile(shape, dtype, name):
            return persist.tile(shape, dtype, tag=name, name=name)

        w_wcT = ptile([E, 2, 128], dt.bfloat16, "w_wcT")
        w_wcb = ptile([128, 2], dt.float32, "w_wcb")
        w_wihT = ptile([128, 2, 2, 3 * H], dt.bfloat16, "w_wihT")
        w_whhT = ptile([128, 2, 3 * H], dt.bfloat16, "w_whhT")
        b_ih = ptile([128, 2, 3], dt.float32, "b_ih")
        b_hh = ptile([128, 2, 3], dt.float32, "b_hh")
        b_xw = ptile([128, 2, 3], dt.float32, "b_xw")
        w_h2lT = ptile([128, 2, 1], dt.float32, "w_h2lT")
        w_h2lb = ptile([1, L], dt.float32, "w_h2lb")
        pl_sb = ptile([NLANE, 1], dt.int32, "pl_sb")

        idxAr = ptile([128, ntokc // 16], dt.int16, "idxAr")
        idxBr = ptile([128, ntokc // 16], dt.int16, "idxBr")
        dc_sb = ptile([128, NSTMT // 16], dt.int16, "dc_sb")

        ident = ptile([128, 128], dt.bfloat16, "ident")
        smask = ptile([128, NSTMT], dt.bfloat16, "smask")
        stmt_pre = ptile([128, 2, nvc], dt.float32, "stmt_pre")
        encC = ptile([128, 2, nvc], dt.float32, "encC")
        encT = ptile([128, 2, NSTMT], dt.bfloat16, "encT")
        # xw per step: 64 cols = [r_f8 r_b8 | z_f8 z_b8 | (bhh_n,xw_n)x16]
        # (backward direction is stored step-reversed; the n-gate block is
        # interleaved (bhh_n, xw_n) pairs per lane for the psum inject; the
        # z block is NEGATED -- z-gate weights/bias are negated host-side so
        # one sigmoid yields [r | 1-z] at once)
        xw48 = ptile([128, S, 8 * NLANE], dt.bfloat16, "xw48")
        # h stored interleaved: [., t, lane, 1] = h; [., t, lane, 0] = n junk
        outbuf = ptile([128, S + 1, 2 * NLANE, 2], dt.bfloat16, "outbuf")
        # [0, r]x16 pairs then [0, zc]x16 pairs (evens stay zero)
        rzpat0 = ptile([128, 2 * W2, 2], dt.float32, "rzpat0")
        rzpat1 = ptile([128, 2 * W2, 2], dt.float32, "rzpat1")
        bn_z = ptile([128, 2], dt.float32, "bn_z")
        mxall = ptile([128, W2], dt.float32, "mxall")
        ad = ptile([128, 2, PB], dt.float32, "ad")
        probs_sb = ptile([1, 2 * PB], dt.float32, "probs_sb")

        # ---------------- weight / input loads ----------------
        nc.sync.dma_start(w_wcT[:], wcT[:])
        nc.sync.dma_start(w_wcb[:], wcb[:])
        nc.sync.dma_start(w_wihT[:], wihT[:].rearrange("d k p g -> p d k g"))
        nc.sync.dma_start(w_whhT[:], whhT[:].rearrange("d p g -> p d g"))
        nc.sync.dma_start(b_ih[:], bih3[:].rearrange("d p g -> p d g"))
        nc.sync.dma_start(b_hh[:], bhh3[:].rearrange("d p g -> p d g"))
        nc.sync.dma_start(w_h2lT[:], h2lT[:].rearrange("k p l -> p k l"))
        nc.sync.dma_start(w_h2lb[:], h2lb[:])
        nc.sync.dma_start(pl_sb[:], plens[:].rearrange("(p o) -> p o", o=1))
        nc.sync.dma_start(idxAr[:], idxag[:])
        nc.sync.dma_start(idxBr[:], idxbg[:])
        nc.sync.dma_start(dc_sb[:], dcidx[:])

        make_identity(nc, ident[:])

        # combined bias for the xw fold: r/z get bih+bhh, n gets bih only
        nc.vector.tensor_tensor(b_xw[:], b_ih[:], b_hh[:], Alu.add)
        nc.vector.tensor_copy(b_xw[:, :, 2], b_ih[:, :, 2])
        # negated z bias (the z preact is stored negated)
        nc.vector.tensor_scalar_mul(bn_z[:], b_xw[:, :, 1], -1.0)

        # ---------------- statement-validity mask ----------------
        scratch = ctx.enter_context(tc.tile_pool(name="scratch", bufs=1))
        # msp[lane, s] = (plen[lane] >= S - s); bounce through DRAM to
        # replicate across the 128 D-partitions.
        thr = scratch.tile([NLANE, S], dt.int32, tag="thr")
        nc.gpsimd.iota(thr[:], pattern=[[-1, S]], base=S, channel_multiplier=0)
        thr_f = scratch.tile([NLANE, S], dt.float32, tag="thr_f")
        nc.vector.tensor_copy(thr_f[:], thr[:])
        pl_f = scratch.tile([NLANE, 1], dt.float32, tag="pl_f")
        nc.vector.tensor_copy(pl_f[:], pl_sb[:])
        msp_bf = scratch.tile([NLANE, S], dt.bfloat16, tag="msp_bf")
        nc.vector.tensor_single_scalar(out=msp_bf[:], in_=thr_f[:],
                                       scalar=pl_f[:], op=Alu.is_le)
        smask_d = dram.tile([NLANE, S], dt.bfloat16, tag="smask_d")
        nc.sync.dma_start(smask_d[:], msp_bf[:])
        nc.sync.dma_start(
            smask[:],
            smask_d[:].rearrange("l s -> (l s)").unsqueeze(0)
            .broadcast_to([128, NSTMT]))

        # ---------------- embed: gather + Wc + token-max (compacted) ----
        with tc.tile_pool(name="gx", bufs=3) as gx, \
             tc.tile_pool(name="pemb", bufs=6, space="PSUM") as pemb:
            qa, qb = (gq if gq is not None else (CTOK, CTOK))
            for j in range(nchunk):
                xa = gx.tile([128, 1, CTOK], dt.bfloat16, tag="xa")
                xb = gx.tile([128, 1, CTOK], dt.bfloat16, tag="xb")
                gsl = slice((CTOK // 16) * j, (CTOK // 16) * (j + 1))
                nc.gpsimd.dma_gather(
                    out_ap=xa[:], in_ap=ptab[0:SPLIT + 1, :],
                    idxs_ap=idxAr[:, gsl],
                    num_idxs=CTOK, num_idxs_reg=qa, elem_size=E,
                    transpose=True, single_packet=False)
                nc.gpsimd.dma_gather(
                    out_ap=xb[:], in_ap=ptab[SPLIT + 1:PTAB_ROWS, :],
                    idxs_ap=idxBr[:, gsl],
                    num_idxs=CTOK, num_idxs_reg=qb, elem_size=E,
                    transpose=True, single_packet=False)
                if cut == "gather":
                    continue
                for dh in range(2):
                    for mt in range(CTOK // 512):
                        ps = pemb.tile([128, 512], dt.float32, tag="pe")
                        sl = slice(512 * mt, 512 * (mt + 1))
                        nc.tensor.matmul(ps[:], w_wcT[:, dh, :],
                                         xa[:, 0, sl], start=True, stop=False)
                        nc.tensor.matmul(ps[:], w_wcT[:, dh, :],
                                         xb[:, 0, sl], start=False, stop=True)
                        c0 = j * (CTOK // T) + mt * 16
                        nc.vector.tensor_reduce(
                            out=stmt_pre[:, dh, c0:c0 + 16],
                            in_=ps[:].rearrange("p (g t) -> p g t", t=T),
                            axis=mybir.AxisListType.X, op=Alu.max)

        # ------- enc = smask * decompact(tanh(pre + bias)) -------
        if cut != "gather":
            for dh in range(2):
                nc.scalar.activation(encC[:, dh, :], stmt_pre[:, dh, :],
                                     Act.Tanh, bias=w_wcb[:, dh:dh + 1],
                                     scale=1.0)
                encF = scratch.tile([128, NSTMT], dt.float32, tag=f"encF{dh}")
                nc.gpsimd.ap_gather(
                    out_ap=encF[:].rearrange("p (v o) -> p v o", o=1),
                    in_ap=encC[:, dh, :].rearrange("p (v o) -> p v o", o=1),
                    idxs_ap=dc_sb[:], channels=128, num_elems=nvc, d=1,
                    num_idxs=NSTMT)
                nc.vector.tensor_tensor(encT[:, dh, :], encF[:], smask[:],
                                        Alu.mult)

        # ---------------- xw precompute ----------------
        # encT cols are lane-major: col = lane*S + s
        with tc.tile_pool(name="pxw", bufs=4, space="PSUM") as pxw:
          if cut not in ("embed", "gather"):
            for d in range(2):
                for g in range(3):
                    for n2 in range(NSTMT // 512):
                        ps = pxw.tile([128, 512], dt.float32, tag="pxw")
                        for kb in range(2):
                            nc.tensor.matmul(
                                ps[:],
                                w_wihT[:, d, kb, g * H:(g + 1) * H],
                                encT[:, kb, 512 * n2:512 * (n2 + 1)],
                                start=(kb == 0), stop=(kb == 1))
                        # psum cols = (lane, s) lane-major; lanes n2*4..n2*4+4
                        l0 = d * NLANE + 4 * n2
                        if g < 2:
                            dst = xw48[:, :, g * W2 + l0:g * W2 + l0 + 4]
                        else:
                            # n gate: odd cols of the interleaved pair block
                            dst = xw48[:, :, 2 * W2:4 * W2].rearrange(
                                "p s (l two) -> p s l two", two=2)[
                                :, :, l0:l0 + 4, 1]
                        if d == 1:   # backward direction: store s-reversed
                            dst = dst[:, ::-1, :]
                        dst = dst.transpose([0, 2, 1])  # iterate (lane, s)
                        src = ps[:].rearrange("p (l s) -> p l s", s=S)
                        if g in (0, 2):
                            nc.scalar.activation(dst, src, Act.Identity,
                                                 bias=b_xw[:, d, g:g + 1],
                                                 scale=1.0)
                        else:
                            # z gate stored negated: -(xw_z + b_z), on DVE
                            # to unload ACT during the xw phase
                            nc.vector.tensor_scalar(
                                out=dst, in0=src,
                                scalar1=b_xw[:, d, 1:2], scalar2=-1.0,
                                op0=Alu.add, op1=Alu.mult)
            # bhh_n broadcast into the even cols of the n-gate block
            for d in range(2):
                pv = xw48[:, :, 2 * W2:4 * W2].rearrange(
                    "p s (l two) -> p s l two", two=2)
                nc.scalar.activation(pv[:, :, d * NLANE:(d + 1) * NLANE, 0],
                                     pv[:, :, d * NLANE:(d + 1) * NLANE, 1],
                                     Act.Identity, bias=b_hh[:, d, 2:3],
                                     scale=0.0)

        # ---------------- GRU: 128 sequential steps ----------------
        # Both directions run in one instruction stream: 16 lanes
        # (fwd 0:8, bwd 8:16).  Per step the psum holds
        #   [r-preact 16 | z-preact 16 | (pn', xw_n) interleaved 32]
        # with pn' = Whh_n h + bhh_n.  One DVE scan computes
        # y = r*pn' + xw_n (pairwise reset via op0=mult with 0), tanh gives
        # n written into the step's junk column, an in-place subtract turns
        # it into n-h, and a second scan computes h' = h + zc*(n-h)
        # (zc = 1-z straight from a scale=-1 sigmoid).  The running
        # residual+time-max is folded in every BLK steps so no tail
        # reduction over the full history is needed.
        nc.vector.memset(outbuf[:, 0, :, :], 0.0)
        nc.vector.memset(rzpat0[:], 0.0)
        nc.vector.memset(rzpat1[:], 0.0)
        nc.vector.memset(mxall[:], -3e38)
        BLK = 8
        with tc.tile_pool(name="pgru", bufs=3, space="PSUM") as pgru, \
             tc.tile_pool(name="gsb", bufs=3) as gsb:
          if cut not in ("embed", "xw", "gather"):
            NL = NLANE
            nsteps = int(os.environ.get("BPCC_GRUS", S))
            for t in range(nsteps):
                rzpat = rzpat0 if t % 2 == 0 else rzpat1
                hf = outbuf[:, t, 0:NL, 1]
                hb = outbuf[:, t, NL:W2, 1]
                pp = pgru.tile([128, 4 * W2], dt.float32, tag="pp")
                pnv = pp[:, 2 * W2:4 * W2].rearrange(
                    "p (l two) -> p l two", two=2)
                # one inject opens all three regions (r | -z | (bhh,xw_n));
                # each region is closed by its own Whh matmuls
                nc.tensor.matmul(pp[:], ident[:], xw48[:, t, :],
                                 start=True, stop=False)
                nc.tensor.matmul(pp[:, 0:NL], w_whhT[:, 0, 0:H], hf,
                                 start=False, stop=False)
                nc.tensor.matmul(pp[:, NL:W2], w_whhT[:, 1, 0:H], hb,
                                 start=False, stop=False)
                nc.tensor.matmul(pp[:, W2:W2 + NL], w_whhT[:, 0, H:2 * H],
                                 hf, start=False, stop=False)
                nc.tensor.matmul(pp[:, W2 + NL:2 * W2], w_whhT[:, 1, H:2 * H],
                                 hb, start=False, stop=False)
                nc.tensor.matmul(pnv[:, 0:NL, 0], w_whhT[:, 0, 2 * H:3 * H],
                                 hf, start=False, stop=False)
                nc.tensor.matmul(pnv[:, NL:W2, 0], w_whhT[:, 1, 2 * H:3 * H],
                                 hb, start=False, stop=True)
                # one sigmoid: [r | 1-z] -> rzpat odds (z block is negated)
                nc.scalar.activation(rzpat[:, :, 1], pp[:, 0:2 * W2],
                                     Act.Sigmoid)
                # y scan: odd cols get r*pn' + xw_n
                yb = pgru.tile([128, 2 * W2], dt.float32, tag="yb")
                nc.vector.tensor_tensor_scan(
                    yb[:], rzpat[:, 0:W2, :].rearrange("p l two -> p (l two)"),
                    pp[:, 2 * W2:4 * W2], 0.0, op0=Alu.mult, op1=Alu.add)
                ybv = yb[:].rearrange("p (l two) -> p l two", two=2)
                # n into the junk column of step t, then in-place n-h
                nc.scalar.activation(outbuf[:, t, :, 0], ybv[:, :, 1],
                                     Act.Tanh)
                nc.vector.tensor_tensor(outbuf[:, t, :, 0],
                                        outbuf[:, t, :, 0],
                                        outbuf[:, t, :, 1], Alu.subtract)
                # blend scan: odd cols get h + zc*(n-h) = h'
                nc.vector.tensor_tensor_scan(
                    outbuf[:, t + 1, :, :].rearrange("p l two -> p (l two)"),
                    rzpat[:, W2:2 * W2, :].rearrange("p l two -> p (l two)"),
                    outbuf[:, t, :, :].rearrange("p l two -> p (l two)"),
                    0.0, op0=Alu.mult, op1=Alu.add)
                # in-loop residual + time-max every BLK steps (consumers
                # only -- fills DVE gaps, no serial-chain impact)
                if t % BLK == BLK - 1 and cut == "":
                    t0 = t + 1 - BLK
                    g8 = gsb.tile([128, BLK, W2], dt.float32, tag="g8")
                    for dh in range(2):
                        ebase = encT[:, dh, :].rearrange(
                            "p (l s) -> p s l", s=S)
                        if dh == 1:
                            ebase = ebase[:, ::-1, :]
                        nc.vector.tensor_tensor(
                            g8[:, :, dh * NL:(dh + 1) * NL],
                            outbuf[:, t0 + 1:t0 + 1 + BLK,
                                   dh * NL:(dh + 1) * NL, 1],
                            ebase[:, t0:t0 + BLK, :], Alu.add)
                    bm = gsb.tile([128, W2], dt.float32, tag="bm")
                    nc.vector.tensor_reduce(
                        out=bm[:], in_=g8[:].transpose([0, 2, 1]),
                        axis=mybir.AxisListType.X, op=Alu.max)
                    nc.vector.tensor_tensor(mxall[:], mxall[:], bm[:],
                                            Alu.max)

        # ---------------- residual + time max-pool + head ----------------
        with tc.tile_pool(name="tail", bufs=1) as tail, \
             tc.tile_pool(name="phead", bufs=1, space="PSUM") as phead:
          if cut in ("embed", "xw", "gru"):
            nc.sync.dma_start(out_d[:], stmt_pre[:PB, 0, 0:L])
          if cut == "":
            for dh in range(2):
                # |lvec - rvec|  (lanes 0..3 = side1, 4..7 = side2);
                # the time-max already lives in mxall from the in-loop fold
                nc.vector.tensor_tensor(
                    ad[:, dh, :], mxall[:, dh * NLANE:dh * NLANE + PB],
                    mxall[:, dh * NLANE + PB:(dh + 1) * NLANE], Alu.subtract)
                nc.scalar.activation(ad[:, dh, :], ad[:, dh, :], Act.Abs)

            # head: only the logit DIFFERENCE matters for the 2-class
            # softmax; the host pre-subtracts the two head rows so this is a
            # single 1-col matmul (result on one partition, no bounce).
            pl = phead.tile([1, PB], dt.float32, tag="ph")
            for dh in range(2):
                nc.tensor.matmul(pl[:], w_h2lT[:, dh, :], ad[:, dh, :],
                                 start=(dh == 0), stop=(dh == 1))
            bd = tail.tile([1, 2], dt.float32, tag="bd")
            nc.vector.tensor_tensor(bd[:, 0:1], w_h2lb[:, 0:1],
                                    w_h2lb[:, 1:2], Alu.subtract)
            nc.vector.tensor_tensor(bd[:, 1:2], w_h2lb[:, 1:2],
                                    w_h2lb[:, 0:1], Alu.subtract)
            # softmax over 2 classes == sigmoid of the logit difference;
            # write interleaved (prog-major) so the output DMA is contiguous
            pview = probs_sb[:].rearrange("o (p l) -> o p l", l=L)
            nc.scalar.activation(pview[:, :, 0], pl[:], Act.Sigmoid,
                                 bias=bd[:, 0:1], scale=1.0)
            nc.scalar.activation(pview[:, :, 1], pl[:], Act.Sigmoid,
                                 bias=bd[:, 1:2], scale=-1.0)
            nc.sync.dma_start(out_d[:].rearrange("p l -> (p l)").unsqueeze(0),
                              probs_sb[:])

    nc.compile()
    return nc


def _get_program(nvc=NSTMT, gq=None):
    key = ("nc", nvc, gq)
    if key not in _cache:
        _cache[key] = _build_program(nvc, gq)
    return _cache[key]


# ------------------------------------------------------------- host side ----
def _prep_shared(inputs):
    emb = np.asarray(inputs["emb"], np.float32)
    pt = np.zeros((PTAB_ROWS, E), dtype=BF16)
    pt[1:SPLIT + 1] = emb[:SPLIT].astype(BF16)
    pt[SPLIT + 2:] = emb[SPLIT:].astype(BF16)

    wcT = np.ascontiguousarray(
        np.asarray(inputs["Wc_w"], np.float32).T).astype(BF16)
    wcT = wcT.reshape(E, 2, 128)
    wcb = np.ascontiguousarray(
        np.asarray(inputs["Wc_b"], np.float32).reshape(2, 128).T)

    wihT = np.stack([np.asarray(inputs[k], np.float32).T
                     for k in ("wih_f", "wih_b")])
    wihT = np.ascontiguousarray(wihT.reshape(2, 2, 128, 3 * H)).astype(BF16)
    whhT = np.stack([np.asarray(inputs[k], np.float32).T
                     for k in ("whh_f", "whh_b")])
    whhT[:, :, 128:256] *= -1.0     # z-gate negated (see device comment)
    whhT = np.ascontiguousarray(whhT).astype(BF16)          # [2, 128, 384]
    bih3 = np.ascontiguousarray(np.stack(
        [np.asarray(inputs[k], np.float32).reshape(3, 128).T
         for k in ("bih_f", "bih_b")]))
    bhh3 = np.ascontiguousarray(np.stack(
        [np.asarray(inputs[k], np.float32).reshape(3, 128).T
         for k in ("bhh_f", "bhh_b")]))
    h2ld = np.asarray(inputs["h2l_w"], np.float32)
    h2lT = np.ascontiguousarray(
        (h2ld[0] - h2ld[1]).reshape(2, 128, 1))
    h2lb = np.asarray(inputs["h2l_b"], np.float32).reshape(1, L)
    return dict(ptab=pt, wcT=wcT, wcb=wcb, wihT=wihT, whhT=whhT,
                bih3=bih3, bhh3=bhh3, h2lT=h2lT, h2lb=h2lb)


def _wrap16(flat, width):
    """[n] int -> [128, n//16]: idx i of 4096-chunk j sits at
    [(i%16) + 16k, (CTOK//16)*j + i//16] for every partition group k."""
    n = flat.shape[0]
    out = np.zeros((128, n // 16), np.int16)
    for j in range(n // width):
        blk = flat[width * j:width * (j + 1)].reshape(width // 16, 16).T
        for k in range(8):
            out[16 * k:16 * (k + 1), (width // 16) * j:
                (width // 16) * (j + 1)] = blk
    return out


def _balance(inputs):
    """Assign program pairs to cores, balancing total valid statements."""
    w = (np.asarray(inputs["prog_lens1"]).astype(np.int64) +
         np.asarray(inputs["prog_lens2"]).astype(np.int64))
    order = np.argsort(-w)
    sums = [0] * NCORES
    buckets = [[] for _ in range(NCORES)]
    for i in order:
        open_cores = [j for j in range(NCORES) if len(buckets[j]) < PB]
        c = min(open_cores, key=lambda j: sums[j])
        buckets[c].append(int(i))
        sums[c] += int(w[i])
    perm = [i for b in buckets for i in b]
    nvc = -(-max(sums) // 128) * 128
    return perm, nvc


def _prep_core(c, inputs, perm, nvc):
    progs = perm[PB * c:PB * (c + 1)]
    tk = np.stack([np.asarray(inputs["tokens1"])[progs],
                   np.asarray(inputs["tokens2"])[progs]])
    tk = tk.astype(np.int64).reshape(NLANE, S, T)       # lane = side*PB+prog
    tl = np.stack([np.asarray(inputs["token_lens1"])[progs],
                   np.asarray(inputs["token_lens2"])[progs]])
    tl = tl.astype(np.int64).reshape(NLANE, S)
    pl = np.stack([np.asarray(inputs["prog_lens1"])[progs],
                   np.asarray(inputs["prog_lens2"])[progs]])
    pl = pl.astype(np.int32).reshape(NLANE)

    # compacted valid-statement list (lane-major, s ascending)
    lanes, ss = np.nonzero(np.arange(S)[None, :] >= (S - pl[:, None]))
    nv = lanes.shape[0]
    # effective tokens with invalid slots re-reading the first token
    tkv = tk[lanes, ss]                                 # [nv, T]
    tlv = tl[lanes, ss]                                 # [nv]
    valid = np.arange(T)[None, :] < tlv[:, None]
    eff = np.where(valid, tkv, tkv[:, 0:1])             # [nv, T]
    effp = np.zeros((nvc, T), np.int64)
    effp[:nv] = eff
    vslot = np.arange(nvc)[:, None] < nv
    if _negidx():
        # tokens not belonging to a pass get -1: the SWDGE emits no
        # descriptor for them (the DMA replays the table's row 0 = the zero
        # row, so xa+xb is unchanged).  num_idxs_reg must match the exact
        # non-negative count, so _make_in_maps later pads counts up to a
        # shared per-pass quota by flipping -1 -> 0 (benign zero-row reads).
        idxa = np.where((effp < SPLIT) & vslot, effp + 1, -1)
        idxb = np.where((effp >= SPLIT) & vslot, effp - (SPLIT - 1), -1)
    else:
        idxa = np.where((effp < SPLIT) & vslot, effp + 1, 0)
        idxb = np.where((effp >= SPLIT) & vslot, effp - (SPLIT - 1), 0)
    # decompaction: dense (lane, s) -> compact slot (invalid -> 0, masked)
    dc = np.zeros(NSTMT, np.int64)
    dc[lanes * S + ss] = np.arange(nv)

    return dict(idxag=idxa.reshape(nvc * T),
                idxbg=idxb.reshape(nvc * T),
                dcidx=_wrap16(dc, NSTMT),
                plens=np.ascontiguousarray(pl))


def _negidx():
    # Negative-index descriptor skipping works in CoreSim but crashes real
    # HW (NRT_EXEC_UNIT_UNRECOVERABLE) -- keep disabled.
    return os.environ.get("BPCC_NEGIDX", "0") == "1"


def _make_in_maps(inputs):
    perm, nvc = _balance(inputs)
    shared = _prep_shared(inputs)
    raw = [_prep_core(c, inputs, perm, nvc) for c in range(NCORES)]
    gq = None
    if _negidx():
        # shared per-pass quotas: max valid count over (core, chunk), padded
        # to a multiple of 16; counts are then raised to the quota exactly by
        # flipping -1 -> 0 (extra zero-row reads)
        nchunk = nvc * T // CTOK
        quotas = []
        for key in ("idxag", "idxbg"):
            cnt = max(int((m[key].reshape(nchunk, CTOK)[j] >= 0).sum())
                      for m in raw for j in range(nchunk))
            quotas.append(min(-(-cnt // 16) * 16, CTOK))
        gq = tuple(quotas)
        for m in raw:
            for q, key in zip(gq, ("idxag", "idxbg")):
                idx = m[key].reshape(nchunk, CTOK)
                for j in range(nchunk):
                    neg = np.nonzero(idx[j] < 0)[0]
                    need = q - (CTOK - neg.size)
                    idx[j, neg[:need]] = 0
    in_maps = []
    for c in range(NCORES):
        m = dict(raw[c])
        m["idxag"] = _wrap16(m["idxag"], CTOK)
        m["idxbg"] = _wrap16(m["idxbg"], CTOK)
        m.update(shared)
        in_maps.append(m)
    return in_maps, perm, nvc, gq


def kernel(**inputs):
    from concourse import bass_utils

    in_maps, perm, nvc, gq = _make_in_maps(inputs)
    nc = _get_program(nvc, gq)
    res = bass_utils.run_bass_kernel_spmd(nc, in_maps,
                                          core_ids=list(range(NCORES)))
    kernel.last_results = res
    out = np.concatenate([res.results[c]["probs"] for c in range(NCORES)],
                         axis=0)                        # [B, L] permuted
    full = np.zeros((B, L), np.float32)
    full[np.asarray(perm)] = out.astype(np.float32)
    return np.ascontiguousarray(full.reshape(B, L, 1))


kernel.last_results = None



# revision 32
# speedup vs baseline: 1.5440x; 1.1988x over previous
"""Trainium2 Bass kernel for nn_BatchProgramCC (siamese program classifier).

Network (per side): embed tokens -> per-statement conv (Wc) + tanh + masked
max over tokens -> bidirectional GRU over statements -> residual -> max over
time. Head: softmax(h2l @ |lvec - rvec|).

Distribution: pure data-parallel over the batch (B=32) across 8 NeuronCores
(4 program-pairs per core); weights/embedding replicated. Each core runs an
identical NEFF on its own batch shard; the host concatenates the 8 output
shards. No collectives.

Device-side dataflow per core:
  * embedding gather via the DMA-gather-transpose path (two passes over a
    zero-row-padded bf16 table to cover V=50000 with int16 indices); invalid
    tokens re-gather the statement's first token so the per-statement max is
    unchanged (no -inf masking needed anywhere).
  * Wc projection as bf16 matmuls (E on partitions), per-statement token max
    via grouped reduce_max straight out of PSUM, tanh(+bias) on ACT,
    statement-validity mask multiply.
  * xw = Wih @ enc precomputed for all steps/gates; the GRU runs 128
    sequential steps with fwd+bwd and both sides merged into one 16-lane
    chain ([H on partitions, lanes on free]); per step an identity-matmul
    injects xw into PSUM, 6 small matmuls accumulate Whh h, sigmoid/tanh on
    ACT, gate algebra entirely on DVE via two pairwise scans: tanh lands in
    the step's junk column, an in-place subtract turns it into n-h, and the
    blend scan computes h' = h + (1-z)(n-h) (no GPSIMD in the serial path).
  * the residual + time-max-pool is folded into the GRU loop as an 8-step
    block add/reduce/max on DVE (fills idle gaps, no tail-time mega
    reduction); |l - r|, 2-class softmax via sigmoid.
"""

import os
import numpy as np
import ml_dtypes

# ---------------------------------------------------------------- sizes ----
V, E, D, H, L = 50000, 128, 256, 128, 2
B, S, T = 32, 128, 32
NCORES = 8
PB = B // NCORES            # programs per core = 4
NLANE = 2 * PB              # sequences per direction per core = 8 (side-major)
NSTMT = NLANE * S           # statements per core = 1024
NTOK = NSTMT * T            # tokens per core = 32768
W2 = 2 * NLANE              # lanes across both directions = 16
SPLIT = 32766               # ids < SPLIT go to gather pass A
PTAB_ROWS = 2 + V           # 50002 (two zero rows)
RTAB_ROWS = 32768           # per-core remapped table rows (remap mode)
NCHUNK = 8                  # gather chunks
CTOK = NTOK // NCHUNK       # tokens per chunk = 4096

BF16 = ml_dtypes.bfloat16

_cache = {}


# ------------------------------------------------------------ device IR ----
def _build_program(nvc=NSTMT, gq=None):
    from contextlib import ExitStack
    import concourse.mybir as mybir
    import concourse.tile as tile
    from concourse import bacc
    from concourse.masks import make_identity

    dt = mybir.dt
    Alu = mybir.AluOpType
    Act = mybir.ActivationFunctionType

    nc = bacc.Bacc("TRN2", target_bir_lowering=False, debug=False,
                   num_devices=NCORES)

    ntokc = nvc * T          # compacted token count (statement-compacted)
    nchunk = ntokc // CTOK   # gather chunks of 4096 tokens / 128 statements

    remap = _remap()
    # ---- DRAM tensors (per-core views; same names on every core) ----
    # remap mode: per-core deduped table (row 0 = zeros, rows 1..nuniq =
    # the unique embeddings this core touches; nuniq <= ntokc < 32767 so a
    # single int16 gather pass covers everything)
    ptab = nc.dram_tensor("ptab", [RTAB_ROWS if remap else PTAB_ROWS, E],
                          dt.bfloat16, kind="ExternalInput")
    idxag = nc.dram_tensor("idxag", [128, ntokc // 16], dt.int16,
                           kind="ExternalInput")
    if not remap:
        idxbg = nc.dram_tensor("idxbg", [128, ntokc // 16], dt.int16,
                               kind="ExternalInput")
    dcidx = nc.dram_tensor("dcidx", [128, NSTMT // 16], dt.int16,
                           kind="ExternalInput")
    plens = nc.dram_tensor("plens", [NLANE], dt.int32, kind="ExternalInput")
    wcT = nc.dram_tensor("wcT", [E, 2, 128], dt.bfloat16, kind="ExternalInput")
    wcb = nc.dram_tensor("wcb", [128, 2], dt.float32, kind="ExternalInput")
    wihT = nc.dram_tensor("wihT", [2, 2, 128, 3 * H], dt.bfloat16, kind="ExternalInput")
    whhT = nc.dram_tensor("whhT", [2, H, 3 * H], dt.bfloat16, kind="ExternalInput")
    bih3 = nc.dram_tensor("bih3", [2, 128, 3], dt.float32, kind="ExternalInput")
    bhh3 = nc.dram_tensor("bhh3", [2, 128, 3], dt.float32, kind="ExternalInput")
    h2lT = nc.dram_tensor("h2lT", [2, 128, 1], dt.float32, kind="ExternalInput")
    h2lb = nc.dram_tensor("h2lb", [1, L], dt.float32, kind="ExternalInput")
    out_d = nc.dram_tensor("probs", [PB, L], dt.float32, kind="ExternalOutput")

    cut = os.environ.get("BPCC_CUT", "")
    with tile.TileContext(nc) as tc, ExitStack() as ctx:
        persist = ctx.enter_context(tc.tile_pool(name="persist", bufs=1))
        dram = ctx.enter_context(tc.tile_pool(name="dram", bufs=1, space="DRAM"))

        # ---------------- persistent SBUF buffers ----------------
        def ptile(shape, dtype, name):
            return persist.tile(shape, dtype, tag=name, name=name)

        w_wcT = ptile([E, 2, 128], dt.bfloat16, "w_wcT")
        w_wcb = ptile([128, 2], dt.float32, "w_wcb")
        w_wihT = ptile([128, 2, 2, 3 * H], dt.bfloat16, "w_wihT")
        w_whhT = ptile([128, 2, 3 * H], dt.bfloat16, "w_whhT")
        b_ih = ptile([128, 2, 3], dt.float32, "b_ih")
        b_hh = ptile([128, 2, 3], dt.float32, "b_hh")
        b_xw = ptile([128, 2, 3], dt.float32, "b_xw")
        w_h2lT = ptile([128, 2, 1], dt.float32, "w_h2lT")
        w_h2lb = ptile([1, L], dt.float32, "w_h2lb")
        pl_sb = ptile([NLANE, 1], dt.int32, "pl_sb")

        idxAr = ptile([128, ntokc // 16], dt.int16, "idxAr")
        if not remap:
            idxBr = ptile([128, ntokc // 16], dt.int16, "idxBr")
        dc_sb = ptile([128, NSTMT // 16], dt.int16, "dc_sb")

        ident = ptile([128, 128], dt.bfloat16, "ident")
        smask = ptile([128, NSTMT], dt.bfloat16, "smask")
        stmt_pre = ptile([128, 2, nvc], dt.float32, "stmt_pre")
        encC = ptile([128, 2, nvc], dt.float32, "encC")
        encT = ptile([128, 2, NSTMT], dt.bfloat16, "encT")
        # xw per step, dir-major: [., s, d, 32] = [r8 | z8 | (bhh_n,xw_n)x8]
        # (backward direction is stored step-reversed; the n-gate block is
        # interleaved (bhh_n, xw_n) pairs per lane for the psum inject; the
        # z block is NEGATED -- z-gate weights/bias are negated host-side so
        # one sigmoid yields [r | 1-z] at once)
        xw48 = ptile([128, S, 2, 4 * NLANE], dt.bfloat16, "xw48")
        # h stored interleaved: [., t, lane, 1] = h; [., t, lane, 0] = n junk
        outbuf = ptile([128, S + 1, 2 * NLANE, 2], dt.bfloat16, "outbuf")
        # per dir: [0, r]x8 pairs then [0, zc]x8 pairs (evens stay zero)
        rzpat0 = ptile([128, 2, 2 * NLANE, 2], dt.float32, "rzpat0")
        rzpat1 = ptile([128, 2, 2 * NLANE, 2], dt.float32, "rzpat1")
        bn_z = ptile([128, 2], dt.float32, "bn_z")
        mxall = ptile([128, W2], dt.float32, "mxall")
        ad = ptile([128, 2, PB], dt.float32, "ad")
        probs_sb = ptile([1, 2 * PB], dt.float32, "probs_sb")

        # ---------------- weight / input loads ----------------
        nc.sync.dma_start(w_wcT[:], wcT[:])
        nc.sync.dma_start(w_wcb[:], wcb[:])
        nc.sync.dma_start(w_wihT[:], wihT[:].rearrange("d k p g -> p d k g"))
        nc.sync.dma_start(w_whhT[:], whhT[:].rearrange("d p g -> p d g"))
        nc.sync.dma_start(b_ih[:], bih3[:].rearrange("d p g -> p d g"))
        nc.sync.dma_start(b_hh[:], bhh3[:].rearrange("d p g -> p d g"))
        nc.sync.dma_start(w_h2lT[:], h2lT[:].rearrange("k p l -> p k l"))
        nc.sync.dma_start(w_h2lb[:], h2lb[:])
        nc.sync.dma_start(pl_sb[:], plens[:].rearrange("(p o) -> p o", o=1))
        nc.sync.dma_start(idxAr[:], idxag[:])
        if not remap:
            nc.sync.dma_start(idxBr[:], idxbg[:])
        nc.sync.dma_start(dc_sb[:], dcidx[:])

        make_identity(nc, ident[:])

        # combined bias for the xw fold: r/z get bih+bhh, n gets bih only
        nc.vector.tensor_tensor(b_xw[:], b_ih[:], b_hh[:], Alu.add)
        nc.vector.tensor_copy(b_xw[:, :, 2], b_ih[:, :, 2])
        # negated z bias (the z preact is stored negated)
        nc.vector.tensor_scalar_mul(bn_z[:], b_xw[:, :, 1], -1.0)

        # ---------------- statement-validity mask ----------------
        scratch = ctx.enter_context(tc.tile_pool(name="scratch", bufs=1))
        # msp[lane, s] = (plen[lane] >= S - s); bounce through DRAM to
        # replicate across the 128 D-partitions.
        thr = scratch.tile([NLANE, S], dt.int32, tag="thr")
        nc.gpsimd.iota(thr[:], pattern=[[-1, S]], base=S, channel_multiplier=0)
        thr_f = scratch.tile([NLANE, S], dt.float32, tag="thr_f")
        nc.vector.tensor_copy(thr_f[:], thr[:])
        pl_f = scratch.tile([NLANE, 1], dt.float32, tag="pl_f")
        nc.vector.tensor_copy(pl_f[:], pl_sb[:])
        msp_bf = scratch.tile([NLANE, S], dt.bfloat16, tag="msp_bf")
        nc.vector.tensor_single_scalar(out=msp_bf[:], in_=thr_f[:],
                                       scalar=pl_f[:], op=Alu.is_le)
        smask_d = dram.tile([NLANE, S], dt.bfloat16, tag="smask_d")
        nc.sync.dma_start(smask_d[:], msp_bf[:])
        nc.sync.dma_start(
            smask[:],
            smask_d[:].rearrange("l s -> (l s)").unsqueeze(0)
            .broadcast_to([128, NSTMT]))

        # ---------------- embed: gather + Wc + token-max (compacted) ----
        with tc.tile_pool(name="gx", bufs=3) as gx, \
             tc.tile_pool(name="pemb", bufs=6, space="PSUM") as pemb:
            qa, qb = (gq if gq is not None else (CTOK, CTOK))
            for j in range(nchunk):
                xa = gx.tile([128, 1, CTOK], dt.bfloat16, tag="xa")
                gsl = slice((CTOK // 16) * j, (CTOK // 16) * (j + 1))
                if remap:
                    nc.gpsimd.dma_gather(
                        out_ap=xa[:], in_ap=ptab[:],
                        idxs_ap=idxAr[:, gsl],
                        num_idxs=CTOK, num_idxs_reg=CTOK, elem_size=E,
                        transpose=True, single_packet=False)
                else:
                    xb = gx.tile([128, 1, CTOK], dt.bfloat16, tag="xb")
                    nc.gpsimd.dma_gather(
                        out_ap=xa[:], in_ap=ptab[0:SPLIT + 1, :],
                        idxs_ap=idxAr[:, gsl],
                        num_idxs=CTOK, num_idxs_reg=qa, elem_size=E,
                        transpose=True, single_packet=False)
                    nc.gpsimd.dma_gather(
                        out_ap=xb[:], in_ap=ptab[SPLIT + 1:PTAB_ROWS, :],
                        idxs_ap=idxBr[:, gsl],
                        num_idxs=CTOK, num_idxs_reg=qb, elem_size=E,
                        transpose=True, single_packet=False)
                if cut == "gather":
                    continue
                for dh in range(2):
                    for mt in range(CTOK // 512):
                        ps = pemb.tile([128, 512], dt.float32, tag="pe")
                        sl = slice(512 * mt, 512 * (mt + 1))
                        nc.tensor.matmul(ps[:], w_wcT[:, dh, :],
                                         xa[:, 0, sl], start=True,
                                         stop=remap)
                        if not remap:
                            nc.tensor.matmul(ps[:], w_wcT[:, dh, :],
                                             xb[:, 0, sl], start=False,
                                             stop=True)
                        c0 = j * (CTOK // T) + mt * 16
                        nc.vector.tensor_reduce(
                            out=stmt_pre[:, dh, c0:c0 + 16],
                            in_=ps[:].rearrange("p (g t) -> p g t", t=T),
                            axis=mybir.AxisListType.X, op=Alu.max)

        # ------- enc = smask * decompact(tanh(pre + bias)) -------
        if cut != "gather":
            for dh in range(2):
                nc.scalar.activation(encC[:, dh, :], stmt_pre[:, dh, :],
                                     Act.Tanh, bias=w_wcb[:, dh:dh + 1],
                                     scale=1.0)
                encF = scratch.tile([128, NSTMT], dt.float32, tag=f"encF{dh}")
                nc.gpsimd.ap_gather(
                    out_ap=encF[:].rearrange("p (v o) -> p v o", o=1),
                    in_ap=encC[:, dh, :].rearrange("p (v o) -> p v o", o=1),
                    idxs_ap=dc_sb[:], channels=128, num_elems=nvc, d=1,
                    num_idxs=NSTMT)
                nc.vector.tensor_tensor(encT[:, dh, :], encF[:], smask[:],
                                        Alu.mult)

        # ---------------- xw precompute ----------------
        # encT cols are lane-major: col = lane*S + s
        with tc.tile_pool(name="pxw", bufs=4, space="PSUM") as pxw:
          if cut not in ("embed", "gather"):
            for d in range(2):
                for g in range(3):
                    for n2 in range(NSTMT // 512):
                        ps = pxw.tile([128, 512], dt.float32, tag="pxw")
                        for kb in range(2):
                            nc.tensor.matmul(
                                ps[:],
                                w_wihT[:, d, kb, g * H:(g + 1) * H],
                                encT[:, kb, 512 * n2:512 * (n2 + 1)],
                                start=(kb == 0), stop=(kb == 1))
                        # psum cols = (lane, s) lane-major; lanes n2*4..n2*4+4
                        l0 = 4 * n2
                        if g < 2:
                            dst = xw48[:, :, d,
                                       g * NLANE + l0:g * NLANE + l0 + 4]
                        else:
                            # n gate: odd cols of the interleaved pair block
                            dst = xw48[:, :, d, 2 * NLANE:4 * NLANE].rearrange(
                                "p s (l two) -> p s l two", two=2)[
                                :, :, l0:l0 + 4, 1]
                        if d == 1:   # backward direction: store s-reversed
                            dst = dst[:, ::-1, :]
                        dst = dst.transpose([0, 2, 1])  # iterate (lane, s)
                        src = ps[:].rearrange("p (l s) -> p l s", s=S)
                        if g in (0, 2):
                            nc.scalar.activation(dst, src, Act.Identity,
                                                 bias=b_xw[:, d, g:g + 1],
                                                 scale=1.0)
                        else:
                            # z gate stored negated: -(xw_z + b_z), on DVE
                            # to unload ACT during the xw phase
                            nc.vector.tensor_scalar(
                                out=dst, in0=src,
                                scalar1=b_xw[:, d, 1:2], scalar2=-1.0,
                                op0=Alu.add, op1=Alu.mult)
            # bhh_n broadcast into the even cols of the n-gate block
            for d in range(2):
                pv = xw48[:, :, d, 2 * NLANE:4 * NLANE].rearrange(
                    "p s (l two) -> p s l two", two=2)
                nc.scalar.activation(pv[:, :, :, 0], pv[:, :, :, 1],
                                     Act.Identity, bias=b_hh[:, d, 2:3],
                                     scale=0.0)

        # ---------------- GRU: 128 sequential steps ----------------
        # Both directions run in one instruction stream: 16 lanes
        # (fwd 0:8, bwd 8:16).  Per step the psum holds
        #   [r-preact 16 | z-preact 16 | (pn', xw_n) interleaved 32]
        # with pn' = Whh_n h + bhh_n.  One DVE scan computes
        # y = r*pn' + xw_n (pairwise reset via op0=mult with 0), tanh gives
        # n written into the step's junk column, an in-place subtract turns
        # it into n-h, and a second scan computes h' = h + zc*(n-h)
        # (zc = 1-z straight from a scale=-1 sigmoid).  The running
        # residual+time-max is folded in every BLK steps so no tail
        # reduction over the full history is needed.
        nc.vector.memset(outbuf[:, 0, :, :], 0.0)
        nc.vector.memset(rzpat0[:], 0.0)
        nc.vector.memset(rzpat1[:], 0.0)
        nc.vector.memset(mxall[:], -3e38)
        BLK = 8
        with tc.tile_pool(name="pgru", bufs=3, space="PSUM") as pgru, \
             tc.tile_pool(name="gsb", bufs=3) as gsb:
          if cut not in ("embed", "xw", "gather"):
            NL = NLANE
            nsteps = int(os.environ.get("BPCC_GRUS", S))
            split = os.environ.get("BPCC_SPLIT", "0") == "1"
            for t in range(nsteps):
                rzpat = rzpat0 if t % 2 == 0 else rzpat1
                if split:
                    # two independent chains (fwd / bwd): each chain's
                    # serial path has 4 matmuls + 16-col gate algebra; the
                    # chains phase-shift and fill each other's engine gaps
                    ppd = [pgru.tile([128, 4 * NL], dt.float32, tag=f"pp{d}",
                                     name=f"pp{d}")
                           for d in range(2)]
                    for d in range(2):
                        nc.tensor.matmul(ppd[d][:], ident[:],
                                         xw48[:, t, d, :],
                                         start=True, stop=False)
                    for d in range(2):
                        hd = outbuf[:, t, d * NL:(d + 1) * NL, 1]
                        nc.tensor.matmul(ppd[d][:, 0:NL],
                                         w_whhT[:, d, 0:H], hd,
                                         start=False, stop=False)
                        nc.tensor.matmul(ppd[d][:, NL:2 * NL],
                                         w_whhT[:, d, H:2 * H], hd,
                                         start=False, stop=False)
                        pnv = ppd[d][:, 2 * NL:4 * NL].rearrange(
                            "p (l two) -> p l two", two=2)
                        nc.tensor.matmul(pnv[:, :, 0],
                                         w_whhT[:, d, 2 * H:3 * H], hd,
                                         start=False, stop=True)
                    for d in range(2):
                        nc.scalar.activation(rzpat[:, d, :, 1],
                                             ppd[d][:, 0:2 * NL], Act.Sigmoid)
                    ybd = [gsb.tile([128, 2 * NL], dt.float32, tag=f"yb{d}",
                                    name=f"yb{d}")
                           for d in range(2)]
                    for d in range(2):
                        nc.vector.tensor_tensor_scan(
                            ybd[d][:],
                            rzpat[:, d, 0:NL, :].rearrange(
                                "p l two -> p (l two)"),
                            ppd[d][:, 2 * NL:4 * NL], 0.0,
                            op0=Alu.mult, op1=Alu.add)
                    for d in range(2):
                        ybv = ybd[d][:].rearrange("p (l two) -> p l two",
                                                  two=2)
                        nc.scalar.activation(
                            outbuf[:, t, d * NL:(d + 1) * NL, 0],
                            ybv[:, :, 1], Act.Tanh)
                    for d in range(2):
                        sl = slice(d * NL, (d + 1) * NL)
                        nc.vector.tensor_tensor(outbuf[:, t, sl, 0],
                                                outbuf[:, t, sl, 0],
                                                outbuf[:, t, sl, 1],
                                                Alu.subtract)
                        nc.vector.tensor_tensor_scan(
                            outbuf[:, t + 1, sl, :].rearrange(
                                "p l two -> p (l two)"),
                            rzpat[:, d, NL:2 * NL, :].rearrange(
                                "p l two -> p (l two)"),
                            outbuf[:, t, sl, :].rearrange(
                                "p l two -> p (l two)"),
                            0.0, op0=Alu.mult, op1=Alu.add)
                else:
                    hf = outbuf[:, t, 0:NL, 1]
                    hb = outbuf[:, t, NL:W2, 1]
                    pp = pgru.tile([128, 8 * NL], dt.float32, tag="pp")
                    ppv = pp[:].rearrange("p (d c) -> p d c", d=2)
                    # one inject opens all regions (r | -z | (bhh,xw_n));
                    # each region is closed by its own Whh matmuls
                    nc.tensor.matmul(pp[:], ident[:],
                                     xw48[:, t, :, :].rearrange(
                                         "p d c -> p (d c)"),
                                     start=True, stop=False)
                    nc.tensor.matmul(ppv[:, 0, 0:NL], w_whhT[:, 0, 0:H], hf,
                                     start=False, stop=False)
                    nc.tensor.matmul(ppv[:, 1, 0:NL], w_whhT[:, 1, 0:H], hb,
                                     start=False, stop=False)
                    nc.tensor.matmul(ppv[:, 0, NL:2 * NL],
                                     w_whhT[:, 0, H:2 * H],
                                     hf, start=False, stop=False)
                    nc.tensor.matmul(ppv[:, 1, NL:2 * NL],
                                     w_whhT[:, 1, H:2 * H],
                                     hb, start=False, stop=False)
                    pnv = ppv[:, :, 2 * NL:4 * NL].rearrange(
                        "p d (l two) -> p d l two", two=2)
                    nc.tensor.matmul(pnv[:, 0, :, 0],
                                     w_whhT[:, 0, 2 * H:3 * H],
                                     hf, start=False, stop=False)
                    nc.tensor.matmul(pnv[:, 1, :, 0],
                                     w_whhT[:, 1, 2 * H:3 * H],
                                     hb, start=False, stop=True)
                    # one sigmoid: [r | 1-z] -> rzpat odds (z negated)
                    nc.scalar.activation(rzpat[:, :, :, 1],
                                         ppv[:, :, 0:2 * NL], Act.Sigmoid)
                    # y scan: odd cols get r*pn' + xw_n
                    yb = pgru.tile([128, 2, 2 * NL], dt.float32, tag="yb")
                    for d in range(2):
                        nc.vector.tensor_tensor_scan(
                            yb[:, d, :], rzpat[:, d, 0:NL, :].rearrange(
                                "p l two -> p (l two)"),
                            ppv[:, d, 2 * NL:4 * NL], 0.0,
                            op0=Alu.mult, op1=Alu.add)
                    ybv = yb[:].rearrange("p d (l two) -> p d l two", two=2)
                    # n into the junk column of step t, then in-place n-h
                    nc.scalar.activation(outbuf[:, t, :, 0], ybv[:, :, :, 1],
                                         Act.Tanh)
                    nc.vector.tensor_tensor(outbuf[:, t, :, 0],
                                            outbuf[:, t, :, 0],
                                            outbuf[:, t, :, 1], Alu.subtract)
                    # blend scan: odd cols get h + zc*(n-h) = h'
                    for d in range(2):
                        sl = slice(d * NL, (d + 1) * NL)
                        nc.vector.tensor_tensor_scan(
                            outbuf[:, t + 1, sl, :].rearrange(
                                "p l two -> p (l two)"),
                            rzpat[:, d, NL:2 * NL, :].rearrange(
                                "p l two -> p (l two)"),
                            outbuf[:, t, sl, :].rearrange(
                                "p l two -> p (l two)"),
                            0.0, op0=Alu.mult, op1=Alu.add)
                # in-loop residual + time-max every BLK steps (consumers
                # only -- fills DVE gaps, no serial-chain impact)
                if t % BLK == BLK - 1 and cut == "":
                    t0 = t + 1 - BLK
                    g8 = gsb.tile([128, BLK, W2], dt.float32, tag="g8")
                    for dh in range(2):
                        ebase = encT[:, dh, :].rearrange(
                            "p (l s) -> p s l", s=S)
                        if dh == 1:
                            ebase = ebase[:, ::-1, :]
                        nc.vector.tensor_tensor(
                            g8[:, :, dh * NL:(dh + 1) * NL],
                            outbuf[:, t0 + 1:t0 + 1 + BLK,
                                   dh * NL:(dh + 1) * NL, 1],
                            ebase[:, t0:t0 + BLK, :], Alu.add)
                    bm = gsb.tile([128, W2], dt.float32, tag="bm")
                    nc.vector.tensor_reduce(
                        out=bm[:], in_=g8[:].transpose([0, 2, 1]),
                        axis=mybir.AxisListType.X, op=Alu.max)
                    nc.vector.tensor_tensor(mxall[:], mxall[:], bm[:],
                                            Alu.max)

        # ---------------- residual + time max-pool + head ----------------
        with tc.tile_pool(name="tail", bufs=1) as tail, \
             tc.tile_pool(name="phead", bufs=1, space="PSUM") as phead:
          if cut in ("embed", "xw", "gru"):
            nc.sync.dma_start(out_d[:], stmt_pre[:PB, 0, 0:L])
          if cut == "":
            for dh in range(2):
                # |lvec - rvec|  (lanes 0..3 = side1, 4..7 = side2);
                # the time-max already lives in mxall from the in-loop fold
                nc.vector.tensor_tensor(
                    ad[:, dh, :], mxall[:, dh * NLANE:dh * NLANE + PB],
                    mxall[:, dh * NLANE + PB:(dh + 1) * NLANE], Alu.subtract)
                nc.scalar.activation(ad[:, dh, :], ad[:, dh, :], Act.Abs)

            # head: only the logit DIFFERENCE matters for the 2-class
            # softmax; the host pre-subtracts the two head rows so this is a
            # single 1-col matmul (result on one partition, no bounce).
            pl = phead.tile([1, PB], dt.float32, tag="ph")
            for dh in range(2):
                nc.tensor.matmul(pl[:], w_h2lT[:, dh, :], ad[:, dh, :],
                                 start=(dh == 0), stop=(dh == 1))
            bd = tail.tile([1, 2], dt.float32, tag="bd")
            nc.vector.tensor_tensor(bd[:, 0:1], w_h2lb[:, 0:1],
                                    w_h2lb[:, 1:2], Alu.subtract)
            nc.vector.tensor_tensor(bd[:, 1:2], w_h2lb[:, 1:2],
                                    w_h2lb[:, 0:1], Alu.subtract)
            # softmax over 2 classes == sigmoid of the logit difference;
            # write interleaved (prog-major) so the output DMA is contiguous
            pview = probs_sb[:].rearrange("o (p l) -> o p l", l=L)
            nc.scalar.activation(pview[:, :, 0], pl[:], Act.Sigmoid,
                                 bias=bd[:, 0:1], scale=1.0)
            nc.scalar.activation(pview[:, :, 1], pl[:], Act.Sigmoid,
                                 bias=bd[:, 1:2], scale=-1.0)
            nc.sync.dma_start(out_d[:].rearrange("p l -> (p l)").unsqueeze(0),
                              probs_sb[:])

    nc.compile()
    return nc


def _get_program(nvc=NSTMT, gq=None):
    key = ("nc", nvc, gq)
    if key not in _cache:
        _cache[key] = _build_program(nvc, gq)
    return _cache[key]


# ------------------------------------------------------------- host side ----
def _remap():
    return os.environ.get("BPCC_REMAP", "1") == "1"


def _prep_shared(inputs):
    if not _remap():
        emb = np.asarray(inputs["emb"], np.float32)
        pt = np.zeros((PTAB_ROWS, E), dtype=BF16)
        pt[1:SPLIT + 1] = emb[:SPLIT].astype(BF16)
        pt[SPLIT + 2:] = emb[SPLIT:].astype(BF16)

    wcT = np.ascontiguousarray(
        np.asarray(inputs["Wc_w"], np.float32).T).astype(BF16)
    wcT = wcT.reshape(E, 2, 128)
    wcb = np.ascontiguousarray(
        np.asarray(inputs["Wc_b"], np.float32).reshape(2, 128).T)

    wihT = np.stack([np.asarray(inputs[k], np.float32).T
                     for k in ("wih_f", "wih_b")])
    wihT = np.ascontiguousarray(wihT.reshape(2, 2, 128, 3 * H)).astype(BF16)
    whhT = np.stack([np.asarray(inputs[k], np.float32).T
                     for k in ("whh_f", "whh_b")])
    whhT[:, :, 128:256] *= -1.0     # z-gate negated (see device comment)
    whhT = np.ascontiguousarray(whhT).astype(BF16)          # [2, 128, 384]
    bih3 = np.ascontiguousarray(np.stack(
        [np.asarray(inputs[k], np.float32).reshape(3, 128).T
         for k in ("bih_f", "bih_b")]))
    bhh3 = np.ascontiguousarray(np.stack(
        [np.asarray(inputs[k], np.float32).reshape(3, 128).T
         for k in ("bhh_f", "bhh_b")]))
    h2ld = np.asarray(inputs["h2l_w"], np.float32)
    h2lT = np.ascontiguousarray(
        (h2ld[0] - h2ld[1]).reshape(2, 128, 1))
    h2lb = np.asarray(inputs["h2l_b"], np.float32).reshape(1, L)
    out = dict(wcT=wcT, wcb=wcb, wihT=wihT, whhT=whhT,
               bih3=bih3, bhh3=bhh3, h2lT=h2lT, h2lb=h2lb)
    if not _remap():
        out["ptab"] = pt
    return out


def _wrap16(flat, width):
    """[n] int -> [128, n//16]: idx i of 4096-chunk j sits at
    [(i%16) + 16k, (CTOK//16)*j + i//16] for every partition group k."""
    n = flat.shape[0]
    out = np.zeros((128, n // 16), np.int16)
    for j in range(n // width):
        blk = flat[width * j:width * (j + 1)].reshape(width // 16, 16).T
        for k in range(8):
            out[16 * k:16 * (k + 1), (width // 16) * j:
                (width // 16) * (j + 1)] = blk
    return out


def _balance(inputs):
    """Assign program pairs to cores, balancing total valid statements."""
    w = (np.asarray(inputs["prog_lens1"]).astype(np.int64) +
         np.asarray(inputs["prog_lens2"]).astype(np.int64))
    order = np.argsort(-w)
    sums = [0] * NCORES
    buckets = [[] for _ in range(NCORES)]
    for i in order:
        open_cores = [j for j in range(NCORES) if len(buckets[j]) < PB]
        c = min(open_cores, key=lambda j: sums[j])
        buckets[c].append(int(i))
        sums[c] += int(w[i])
    perm = [i for b in buckets for i in b]
    nvc = -(-max(sums) // 128) * 128
    return perm, nvc


def _prep_core(c, inputs, perm, nvc):
    progs = perm[PB * c:PB * (c + 1)]
    tk = np.stack([np.asarray(inputs["tokens1"])[progs],
                   np.asarray(inputs["tokens2"])[progs]])
    tk = tk.astype(np.int64).reshape(NLANE, S, T)       # lane = side*PB+prog
    tl = np.stack([np.asarray(inputs["token_lens1"])[progs],
                   np.asarray(inputs["token_lens2"])[progs]])
    tl = tl.astype(np.int64).reshape(NLANE, S)
    pl = np.stack([np.asarray(inputs["prog_lens1"])[progs],
                   np.asarray(inputs["prog_lens2"])[progs]])
    pl = pl.astype(np.int32).reshape(NLANE)

    # compacted valid-statement list (lane-major, s ascending)
    lanes, ss = np.nonzero(np.arange(S)[None, :] >= (S - pl[:, None]))
    nv = lanes.shape[0]
    # effective tokens with invalid slots re-reading the first token
    tkv = tk[lanes, ss]                                 # [nv, T]
    tlv = tl[lanes, ss]                                 # [nv]
    valid = np.arange(T)[None, :] < tlv[:, None]
    eff = np.where(valid, tkv, tkv[:, 0:1])             # [nv, T]
    effp = np.zeros((nvc, T), np.int64)
    effp[:nv] = eff
    vslot = np.arange(nvc)[:, None] < nv
    if _remap():
        # per-core deduped table: one int16 gather pass covers all tokens
        uniq, inv = np.unique(effp, return_inverse=True)
        assert uniq.size <= RTAB_ROWS - 2    # idx = inv+1 must fit int16
        emb = np.asarray(inputs["emb"], np.float32)
        pt = np.zeros((RTAB_ROWS, E), dtype=BF16)
        pt[1:1 + uniq.size] = emb[uniq].astype(BF16)
        dc = np.zeros(NSTMT, np.int64)
        dc[lanes * S + ss] = np.arange(nv)
        return dict(idxag=np.asarray(inv, np.int64).reshape(nvc * T) + 1,
                    ptab=pt, dcidx=_wrap16(dc, NSTMT),
                    plens=np.ascontiguousarray(pl))
    if _negidx():
        # tokens not belonging to a pass get -1: the SWDGE emits no
        # descriptor for them (the DMA replays the table's row 0 = the zero
        # row, so xa+xb is unchanged).  num_idxs_reg must match the exact
        # non-negative count, so _make_in_maps later pads counts up to a
        # shared per-pass quota by flipping -1 -> 0 (benign zero-row reads).
        idxa = np.where((effp < SPLIT) & vslot, effp + 1, -1)
        idxb = np.where((effp >= SPLIT) & vslot, effp - (SPLIT - 1), -1)
    else:
        idxa = np.where((effp < SPLIT) & vslot, effp + 1, 0)
        idxb = np.where((effp >= SPLIT) & vslot, effp - (SPLIT - 1), 0)
    # decompaction: dense (lane, s) -> compact slot (invalid -> 0, masked)
    dc = np.zeros(NSTMT, np.int64)
    dc[lanes * S + ss] = np.arange(nv)

    return dict(idxag=idxa.reshape(nvc * T),
                idxbg=idxb.reshape(nvc * T),
                dcidx=_wrap16(dc, NSTMT),
                plens=np.ascontiguousarray(pl))


def _negidx():
    # Negative-index descriptor skipping works in CoreSim but crashes real
    # HW (NRT_EXEC_UNIT_UNRECOVERABLE) -- keep disabled.
    return os.environ.get("BPCC_NEGIDX", "0") == "1"


def _make_in_maps(inputs):
    perm, nvc = _balance(inputs)
    shared = _prep_shared(inputs)
    raw = [_prep_core(c, inputs, perm, nvc) for c in range(NCORES)]
    gq = None
    if _negidx() and not _remap():
        # shared per-pass quotas: max valid count over (core, chunk), padded
        # to a multiple of 16; counts are then raised to the quota exactly by
        # flipping -1 -> 0 (extra zero-row reads)
        nchunk = nvc * T // CTOK
        quotas = []
        for key in ("idxag", "idxbg"):
            cnt = max(int((m[key].reshape(nchunk, CTOK)[j] >= 0).sum())
                      for m in raw for j in range(nchunk))
            quotas.append(min(-(-cnt // 16) * 16, CTOK))
        gq = tuple(quotas)
        for m in raw:
            for q, key in zip(gq, ("idxag", "idxbg")):
                idx = m[key].reshape(nchunk, CTOK)
                for j in range(nchunk):
                    neg = np.nonzero(idx[j] < 0)[0]
                    need = q - (CTOK - neg.size)
                    idx[j, neg[:need]] = 0
    in_maps = []
    for c in range(NCORES):
        m = dict(raw[c])
        m["idxag"] = _wrap16(m["idxag"], CTOK)
        if "idxbg" in m:
            m["idxbg"] = _wrap16(m["idxbg"], CTOK)
        m.update(shared)
        in_maps.append(m)
    return in_maps, perm, nvc, gq


def kernel(**inputs):
    from concourse import bass_utils

    in_maps, perm, nvc, gq = _make_in_maps(inputs)
    nc = _get_program(nvc, gq)
    res = bass_utils.run_bass_kernel_spmd(nc, in_maps,
                                          core_ids=list(range(NCORES)))
    kernel.last_results = res
    out = np.concatenate([res.results[c]["probs"] for c in range(NCORES)],
                         axis=0)                        # [B, L] permuted
    full = np.zeros((B, L), np.float32)
    full[np.asarray(perm)] = out.astype(np.float32)
    return np.ascontiguousarray(full.reshape(B, L, 1))


kernel.last_results = None

